# revision 65
# baseline (speedup 1.0000x reference)
# Multi-head causal attention (b=2, n=2048, dim=1024, 16 heads) on 8 TRN2
# NeuronCores. Sharding: core c -> batch c//4, head-group c%4 (4 heads = 256
# channels). Host pre-transposes x and the weight slices so every device-side
# matmul contracts over the partition dimension with no on-device transposes.
#
# Device-side layout (per core):
#   xT   [1024, 2048]  x[b].T                  (streamed in 512-col chunks)
#   QTz  [128, 4, 2048] per-head Q^T blocks, off-head rows zeroed (so K=128
#                       contraction with a 2-head-packed K tile picks out one
#                       head and every matmul stays in 128x128 tile mode)
#   KT   [128, 2, 2048] K^T, two heads packed per 128-partition block
#   Vp   [128, 16, 4, 65] V in [j, head, d+1] layout; col 64 = ones so the AV
#                       matmul also produces the softmax denominator (row 64)
#   S^T  computed as [j, i] tiles => attn@V needs no transposes and the
#                       normalizer lands in a psum row.
# Softmax skips max-subtraction (scores are O(+-10); exp is safe in fp32).
# Causal masking: -1e30 added to the invalid region of diagonal score tiles
# (DVE, using right-aligned slices of one precomputed mask tile) before exp;
# fully-invalid column spans are simply never computed (sub-span matmuls).
# All matmuls run in float32r (full PE rate at free-dim >= 256).
import os

if os.environ.get("JAX_PLATFORMS") == "cpu":
    # bass2jax must see the axon/neuron PJRT devices.
    del os.environ["JAX_PLATFORMS"]

from contextlib import ExitStack

import numpy as np

import concourse.bass as bass
import concourse.bacc as bacc
import concourse.mybir as mybir
import concourse.tile as tile
from concourse import bass_utils

F32 = mybir.dt.float32
F32R = mybir.dt.float32r
AF = mybir.ActivationFunctionType

P = 128
SEQ = 2048
DIM = 1024
CH = 256          # channels per core (4 heads x 64)
HD = 64           # head dim
NH = 4            # heads per core
KO = DIM // P     # 8 contraction chunks
NIC = SEQ // 512  # 4 i-chunks
SCALE = float(HD) ** -0.5
MBIG = -1.0e30


def build_nc():
    nc = bacc.Bacc("TRN2", target_bir_lowering=False, debug=False, num_devices=8)
    xT = nc.dram_tensor("xT", [DIM, SEQ], F32R, kind="ExternalInput").ap()
    wqT = nc.dram_tensor("wqT", [DIM, CH], F32R, kind="ExternalInput").ap()
    wkT = nc.dram_tensor("wkT", [DIM, CH], F32R, kind="ExternalInput").ap()
    wvT = nc.dram_tensor("wvT", [DIM, CH], F32R, kind="ExternalInput").ap()
    woT = nc.dram_tensor("woT", [CH, DIM], F32R, kind="ExternalInput").ap()
    oo = nc.dram_tensor("oo", [P, P], F32R, kind="ExternalInput").ap()
    o2 = nc.dram_tensor("o2", [P, HD], F32R, kind="ExternalInput").ap()
    out = nc.dram_tensor("out", [SEQ, DIM], F32, kind="ExternalOutput").ap()

    with ExitStack() as ctx:
        tc = ctx.enter_context(tile.TileContext(nc))
        per = ctx.enter_context(tc.tile_pool(name="persist", bufs=1))
        wpool = ctx.enter_context(tc.tile_pool(name="wts", bufs=1))
        xpool = ctx.enter_context(tc.tile_pool(name="xch", bufs=3))
        espool = ctx.enter_context(tc.tile_pool(name="es", bufs=4))
        spool = ctx.enter_context(tc.tile_pool(name="small", bufs=2))
        opool = ctx.enter_context(tc.tile_pool(name="osb", bufs=3))
        psS = ctx.enter_context(tc.tile_pool(name="psS", bufs=2, space="PSUM"))
        psO = ctx.enter_context(tc.tile_pool(name="psO", bufs=2, space="PSUM"))
        psP = ctx.enter_context(tc.tile_pool(name="psP", bufs=2, space="PSUM"))

        QTz = per.tile([P, NH, SEQ], F32R)
        KT = per.tile([P, 2, SEQ], F32R)
        Vp = per.tile([P, SEQ // P, NH, HD + 1], F32R)
        AT = per.tile([P, 2, SEQ], F32R)
        onesFR = per.tile([P, P], F32R)   # row 0 = ones, rest 0 (bcast matmul)
        rsz = per.tile([P, 512], F32R)    # row 0 = 1/denominator, rest 0
        mfull = per.tile([P, 512], F32)   # f32 staging for the mask build
        mask01 = per.tile([P, 512], F32R)  # right-aligned 0/1 causal mask

        wq_s = wpool.tile([P, KO, CH], F32R)
        wk_s = wpool.tile([P, KO, CH], F32R)
        wv_s = wpool.tile([P, KO, CH], F32R)
        wo_s = wpool.tile([P, 2, DIM], F32R)

        xch = {}

        def load_x(ic, split=1):
            t = xpool.tile([P, KO, 512], F32R, tag="xch", name=f"xch{ic}")
            src = xT[:, ic * 512:(ic + 1) * 512].rearrange("(ko p) f -> p ko f", p=P)
            kstep = KO // split
            for s in range(split):
                nc.sync.dma_start(
                    t[:, s * kstep:(s + 1) * kstep, :],
                    src[:, s * kstep:(s + 1) * kstep, :],
                )
            xch[ic] = t

        # DMA order matters for startup latency: the first projection needs
        # only wq + xch0, quartered so the ko-loop can start on the first
        # quarter.
        wq_src = wqT.rearrange("(ko p) c -> p ko c", p=P)
        for s in range(4):
            nc.sync.dma_start(
                wq_s[:, 2 * s:2 * s + 2, :], wq_src[:, 2 * s:2 * s + 2, :]
            )
        load_x(0, split=4)
        nc.sync.dma_start(wk_s[:], wkT.rearrange("(ko p) c -> p ko c", p=P))
        load_x(1)
        nc.sync.dma_start(wv_s[:], wvT.rearrange("(ko p) c -> p ko c", p=P))
        nc.sync.dma_start(wo_s[:], woT.rearrange("(co p) f -> p co f", p=P))
        # small constants arrive by DMA (ones column of Vp, the ones-row
        # matrix for the broadcast matmul); bulk zero-fills stay on engines.
        nc.sync.dma_start(Vp[:, :, :, HD], o2.rearrange("p (a b) -> p a b", a=16))
        nc.sync.dma_start(onesFR[:], oo[:])

        # f32r tiles cannot be memset (walrus ISA check) -> fill f32 scratch
        # and use dtype-converting copies. The bulk QTz zero-fill runs on the
        # otherwise-idle scalar engine during the DMA phase.
        scr = wpool.tile([P, 1024], F32)
        nc.gpsimd.memset(scr[:], 0.0)
        for hh in range(NH):
            off0 = HD if hh % 2 == 0 else 0
            for q in range(2):
                nc.scalar.copy(
                    QTz[off0:off0 + HD, hh, q * 1024:(q + 1) * 1024],
                    scr[0:HD, :] if off0 == 0 else scr[HD:P, :],
                )
        nc.scalar.copy(rsz[:], scr[:, 0:512])

        # PE warmup: dummy fp32 matmuls on scratch keep the HAM busy window
        # hot while the first DMAs land (fp32 = 4 cyc/row burns ~3.4us).
        wps = psP.tile([P, 512], F32, tag="psP", name="warm")
        for _ in range(4):
            nc.tensor.matmul(
                wps[:], lhsT=scr[:, 0:P], rhs=scr[:, 0:512], start=True, stop=True
            )

        # mask: cols [0,384) = 0; cols [384,512) = 1 where f >= p else 0
        nc.gpsimd.memset(mfull[:, 0:384], 0.0)
        nc.gpsimd.memset(mfull[:, 384:512], 1.0)
        nc.gpsimd.affine_select(
            out=mfull[:, 384:512],
            in_=mfull[:, 384:512],
            compare_op=mybir.AluOpType.is_ge,
            fill=0.0,
            base=0,
            channel_multiplier=-1,
            pattern=[[1, P]],
        )
        nc.vector.tensor_copy(mask01[:], mfull[:])

        def proj_mm_unit(ic, co, w_s, name):
            ps = psP.tile([P, 512], F32, tag="psP", name=f"p{name}{ic}{co}")
            for ko in range(KO):
                nc.tensor.matmul(
                    ps[:],
                    lhsT=w_s[:, ko, co * P:(co + 1) * P],
                    rhs=xch[ic][:, ko, :],
                    start=(ko == 0),
                    stop=(ko == KO - 1),
                )
            return ps

        def _cp(eng):
            return nc.scalar.copy if eng == 'act' else nc.vector.tensor_copy

        def proj_q_evac(ic, co, ps, eng=None):
            cp = _cp(eng)
            i0 = ic * 512
            cp(QTz[0:HD, 2 * co, i0:i0 + 512], ps[0:HD, :])
            cp(QTz[HD:P, 2 * co + 1, i0:i0 + 512], ps[HD:P, :])

        def proj_k_evac(ic, co, ps, eng=None):
            i0 = ic * 512
            _cp(eng)(KT[:, co, i0:i0 + 512], ps[:])

        def proj_v_mm(ic, g):
            ps = psP.tile([P, 512], F32, tag="psP", name=f"pv{ic}{g}")
            for u in range(2):
                for ko in range(KO):
                    nc.tensor.matmul(
                        ps[:, u * 256:(u + 1) * 256],
                        lhsT=xch[ic][:, ko, (2 * g + u) * P:(2 * g + u + 1) * P],
                        rhs=wv_s[:, ko, :],
                        start=(ko == 0),
                        stop=(ko == KO - 1),
                    )
            return ps

        def proj_v_evac(ic, g, ps, eng=None):
            _cp(eng)(
                Vp[:, 4 * ic + 2 * g:4 * ic + 2 * g + 2, :, 0:HD],
                ps[:].rearrange("p (j h d) -> p j h d", j=2, h=NH),
            )

        def proj_fillers(ic):
            st = {}
            units = []
            for co in range(2):
                units.append(lambda co=co: st.__setitem__(
                    ('q', co), proj_mm_unit(ic, co, wq_s, 'q')))
                units.append(lambda co=co: proj_q_evac(ic, co, st.pop(('q', co))))
            for co in range(2):
                units.append(lambda co=co: st.__setitem__(
                    ('k', co), proj_mm_unit(ic, co, wk_s, 'k')))
                units.append(lambda co=co: proj_k_evac(ic, co, st.pop(('k', co))))
            for g in range(2):
                units.append(lambda g=g: st.__setitem__(('v', g), proj_v_mm(ic, g)))
                units.append(lambda g=g: proj_v_evac(ic, g, st.pop(('v', g))))
            return units

        def norm(ic, h, pO):
            with nc.allow_low_precision(reason="f32r rounding needed for bcast matmul"):
                nc.vector.reciprocal(rsz[0:1, :], pO[HD:HD + 1, :])
            pB = psP.tile([P, 512], F32, tag="psP", name=f"pB{ic}{h}")
            nc.tensor.matmul(
                pB[:], lhsT=onesFR[:], rhs=rsz[:], start=True, stop=True
            )
            uB = spool.tile([HD, 512], F32, tag="uB", name=f"uB{ic}{h}")
            nc.scalar.copy(uB[:], pB[0:HD, :])
            hp = (h % 2) * HD
            nc.vector.tensor_mul(
                AT[hp:hp + HD, h // 2, ic * 512:(ic + 1) * 512],
                pO[0:HD, :],
                uB[:],
            )

        def wo_unit(io, fc):
            def emit():
                ps2 = psP.tile([P, 512], F32, tag="psP", name=f"po{io}{fc}")
                for co2 in range(2):
                    nc.tensor.matmul(
                        ps2[:],
                        lhsT=AT[:, co2, io * P:(io + 1) * P],
                        rhs=wo_s[:, co2, fc * 512:(fc + 1) * 512],
                        start=(co2 == 0),
                        stop=(co2 == 1),
                    )
                ob = opool.tile([P, 512], F32, tag="ob", name=f"ob{io}{fc}")
                if io >= 12:
                    # final chunk's units run past the exp stream where the
                    # scalar engine is idle; keeps slot recycling off DVE
                    nc.scalar.copy(ob[:], ps2[:])
                else:
                    nc.vector.tensor_copy(ob[:], ps2[:])
                nc.sync.dma_start(
                    out[io * P:(io + 1) * P, fc * 512:(fc + 1) * 512], ob[:]
                )
            return emit

        def wo_units(ic):
            return [wo_unit(4 * ic + ib, fc) for ib in range(4) for fc in range(2)]

        for co in range(2):
            proj_q_evac(0, co, proj_mm_unit(0, co, wq_s, 'q'), eng='act')
        for co in range(2):
            proj_k_evac(0, co, proj_mm_unit(0, co, wk_s, 'k'), eng='act')
        for g in range(2):
            proj_v_evac(0, g, proj_v_mm(0, g), eng='act')
        load_x(2)
        load_x(3)

        # ---- one continuous S -> exp -> AV pipeline across every (ic, h) ----
        S_units = []
        for ic in range(NIC):
            for h in range(NH):
                for t in range(2 * ic + 2):
                    S_units.append((ic, h, t))
        ic_start = {}
        for i, u in enumerate(S_units):
            ic_start.setdefault(u[0], i)

        es_tiles = {}
        off_tab = {}
        pO_tiles = {}
        proj_fq = []   # projection units: must drain before the next ic
        wo_fq = []     # output-projection units: emit whenever
        delayed = []   # (due_pos, fn)

        def emit_S(ic, h, t):
            co = h // 2
            diag = t >= 2 * ic
            pS = psS.tile([P, 1024], F32, tag="psS", name=f"pS{ic}{h}{t}")
            offs = []
            for u in range(2):
                jb = 2 * t + u
                r = jb - 4 * ic
                off = 0 if r < 0 else min(P * r, 256)
                offs.append(off)
                nc.tensor.matmul(
                    pS[:, u * 512 + off:(u + 1) * 512],
                    lhsT=KT[:, co, jb * P:(jb + 1) * P],
                    rhs=QTz[:, h, ic * 512 + off:(ic + 1) * 512],
                    start=True,
                    stop=True,
                )
            es = espool.tile([P, 1024], F32R, tag="es", name=f"es{ic}{h}{t}")
            if diag:
                for u in range(2):
                    r = 2 * (t - 2 * ic) + u
                    off = offs[u]
                    mw = P if r < 3 else 256
                    nc.scalar.activation(
                        es[:, u * 512 + off:(u + 1) * 512],
                        pS[:, u * 512 + off:(u + 1) * 512],
                        AF.Exp,
                        scale=SCALE,
                    )
                    nc.vector.tensor_mul(
                        es[:, u * 512 + off:u * 512 + off + mw],
                        es[:, u * 512 + off:u * 512 + off + mw],
                        mask01[:, 512 - mw:512],
                    )
            else:
                nc.scalar.activation(es[:], pS[:], AF.Exp, scale=SCALE)
            es_tiles[(ic, h, t)] = es
            off_tab[(ic, h, t)] = offs

        def emit_AV(pos, ic, h, t):
            npairs = 2 * ic + 2
            nmm = 2 * npairs
            key = (ic, h)
            if key not in pO_tiles:
                pO_tiles[key] = psO.tile(
                    [P, 512], F32, tag="psO", name=f"pO{ic}{h}"
                )
            pO = pO_tiles[key]
            es = es_tiles.pop((ic, h, t))
            offs = off_tab.pop((ic, h, t))
            for u in range(2):
                jb = 2 * t + u
                off = offs[u]
                nc.tensor.matmul(
                    pO[0:HD + 1, off:512],
                    lhsT=Vp[:, jb, h, :],
                    rhs=es[:, u * 512 + off:(u + 1) * 512],
                    start=(jb == 0),
                    stop=(jb == nmm - 1),
                )
            if t == npairs - 1:
                def fin(ic=ic, h=h, pO=pO, pos=pos):
                    norm(ic, h, pO)
                    pO_tiles.pop((ic, h))
                    if h == NH - 1:
                        wo_fq.extend((pos + 6, u) for u in wo_units(ic))
                delayed.append((pos + 2, fin))

        def run_due(pos):
            while delayed and delayed[0][0] <= pos:
                delayed.pop(0)[1]()

        npos = len(S_units)
        for pos, (ic, h, t) in enumerate(S_units):
            if t == 0 and h == 0:
                while proj_fq:  # safety: next ic's inputs must exist by now
                    proj_fq.pop(0)()
                if ic + 1 < NIC:
                    proj_fq.extend(proj_fillers(ic + 1))
            emit_S(ic, h, t)
            run_due(pos)
            if pos >= 2:
                emit_AV(pos, *S_units[pos - 2])
            if t == 2 * ic + 1:  # head's S-units done -> slip in fillers
                for _ in range(3):
                    if proj_fq:
                        proj_fq.pop(0)()
                for _ in range(2):
                    if wo_fq and wo_fq[0][0] <= pos:
                        wo_fq.pop(0)[1]()
        emit_AV(npos, *S_units[npos - 2])
        run_due(npos)
        emit_AV(npos + 1, *S_units[npos - 1])
        run_due(npos + 10)
        while wo_fq:
            wo_fq.pop(0)[1]()
        for w in wo_units(3):
            w()

    nc.compile()
    return nc


_NC = None


def get_nc():
    global _NC
    if _NC is None:
        _NC = build_nc()
    return _NC


def _oo_const():
    m = np.zeros((P, P), dtype=np.float32)
    m[0, :] = 1.0
    return m


def make_in_maps(x, Wq, Wk, Wv, Wo):
    x = np.ascontiguousarray(np.asarray(x, dtype=np.float32))
    Wq = np.asarray(Wq, dtype=np.float32)
    Wk = np.asarray(Wk, dtype=np.float32)
    Wv = np.asarray(Wv, dtype=np.float32)
    Wo = np.asarray(Wo, dtype=np.float32)
    in_maps = []
    for c in range(8):
        b, g = divmod(c, 4)
        hs = g * CH
        in_maps.append(
            {
                "xT": np.ascontiguousarray(x[b].T),
                "wqT": np.ascontiguousarray(Wq[hs:hs + CH, :].T),
                "wkT": np.ascontiguousarray(Wk[hs:hs + CH, :].T),
                "wvT": np.ascontiguousarray(Wv[hs:hs + CH, :].T),
                "woT": np.ascontiguousarray(Wo[:, hs:hs + CH].T),
                "oo": _oo_const(),
                "o2": np.ones((P, HD), dtype=np.float32),
            }
        )
    return in_maps


LAST_RESULTS = None


def kernel(x, Wq, Wk, Wv, Wo, trace=False):
    global LAST_RESULTS
    nc = get_nc()
    in_maps = make_in_maps(x, Wq, Wk, Wv, Wo)
    res = bass_utils.run_bass_kernel_spmd(
        nc, in_maps, core_ids=list(range(8)), trace=trace
    )
    LAST_RESULTS = res
    partials = [r["out"] for r in res.results]
    out0 = partials[0] + partials[1] + partials[2] + partials[3]
    out1 = partials[4] + partials[5] + partials[6] + partials[7]
    return np.stack([out0, out1]).astype(np.float32)
